# revision 1
# baseline (speedup 1.0000x reference)
"""Trainium2 Bass kernel for AttentionMLPReduction.

Reference computation (per sample, B=256, L=32, H=4096, E=2048, NH=8, hd=256):
  h    = relu(x @ w_red.T + b_red)                  (B,L,E)
  qkv  = h @ w_in.T + b_in ; q,k,v = split(qkv)
  attn = softmax(q @ k.T / sqrt(hd))  per head      (B,NH,L,L)
  ctx  = attn @ v                                   (B,NH,L,hd) -> (B,L,E)
  attn_output = ctx @ w_out.T + b_out               (B,L,E)
  w_mean = attn.mean(heads); w_norm = w_mean / rowsum  (== w_mean, rows sum to 1)
  pooled = mean_q(w_norm @ attn_output)             (B,E)
  out = sigmoid(mlp(pooled))                        (B,1)

Key algebraic simplifications used here:
  * w_norm == w_mean exactly (rows of w_mean already sum to 1).
  * pooled[b] = u[b] @ attn_output[b] with u[b,l] = mean_q w_mean[b,q,l]
    (mean over q commutes with the matmul).
  * pooled = (u @ ctx) @ w_out.T + b_out_eff where
    b_out_eff = b_out + w_out @ b_in_v  (since sum_l u[b,l] == 1), so the
    (B,L,E) attn_output is never materialized and the w_out GEMM shrinks to M=B.

Sharding: pure data parallel over batch; 32 samples per core, weights
replicated. Weights are pre-transposed/cast to bf16 on the host (sharding-time
layout preparation); all device matmuls run in bf16 with f32 accumulation.

Per-core layouts (transposed-activation convention, partition dim first):
  xT    [H=4096, M=1024] bf16   (M = 32 samples x L=32)
  hT    [E=2048, M]      bf16   in SBUF
  qT,kT [hd*2? per head: [256, M]] bf16 via winT GEMM
  v     [M, E]           bf16   natural orientation (lhsT = hT)
  ctx   [M, E]           bf16
  zT    [E, 32]          bf16   z = u @ ctx per sample
  pooledT [E, 32], o1T [256,32], o2T [128,32], o3T [64,32], outT [1,32]
"""

import os
import numpy as np
import ml_dtypes

import concourse.bass as bass
import concourse.mybir as mybir
import concourse.tile as tile
from concourse import bacc
from concourse.bass_utils import run_bass_kernel_spmd
from concourse.masks import make_identity

BF16 = mybir.dt.bfloat16
F32 = mybir.dt.float32
AF = mybir.ActivationFunctionType

B, L, H, E, NH = 256, 32, 4096, 2048, 8
HD = E // NH  # 256
NCORES = 8
BC = B // NCORES  # 32 samples per core
M = BC * L  # 1024 rows per core
P = 128
KX = H // P  # 32 k-tiles for GEMM1
KE = E // P  # 16 k-tiles for E-contraction GEMMs
MT = M // P  # 8 m-tiles
GS = P // L  # 4 samples per partition-tile

# module-level stash for the last run's HW exec time (ns), if traced
LAST_EXEC_TIME_NS = None


def _install_ntff_hook_shim():
    """antenv.axon_hooks is missing in this container; bass_utils imports it
    when trace=True under axon. Recreate it and register the ctypes-driven
    NRT profile hook from trn_boot if available."""
    import sys
    import types
    try:
        from antenv import axon_hooks  # noqa: F401
        return
    except ImportError:
        pass
    try:
        import antenv
    except ImportError:
        return
    m = types.ModuleType("antenv.axon_hooks")
    m._hook = None
    m.set_axon_ntff_profile_hook = lambda h: setattr(m, "_hook", h)
    m.get_axon_ntff_profile_hook = lambda: m._hook
    sys.modules["antenv.axon_hooks"] = m
    antenv.axon_hooks = m
    try:
        from trn_agent_boot.trn_boot import _ntff_profile_via_ctypes
        hook = _ntff_profile_via_ctypes("/opt/axon/libaxon_pjrt.so")
        if hook is not None:
            m._hook = hook
    except Exception:
        pass


def _build_kernel() -> bass.Bass:
    # KBISECT: 1=GEMM1 only, 2=+GEMM2(v,qk), 3=+attention, 4=+z, 0/absent=full
    bisect = int(os.environ.get("KBISECT", "0"))
    nc = bacc.Bacc(None, target_bir_lowering=False, debug=False)

    # ---- DRAM parameters (per-core shard views) ----
    xT = nc.dram_tensor("xT", [H, M], BF16, kind="ExternalInput")
    wredT = nc.dram_tensor("wredT", [H, E], BF16, kind="ExternalInput")
    winT = nc.dram_tensor("winT", [E, 3 * E], BF16, kind="ExternalInput")
    woutT = nc.dram_tensor("woutT", [E, E], BF16, kind="ExternalInput")
    w1T = nc.dram_tensor("w1T", [E, 256], BF16, kind="ExternalInput")
    w2T = nc.dram_tensor("w2T", [256, P], BF16, kind="ExternalInput")
    w3T = nc.dram_tensor("w3T", [P, 64], BF16, kind="ExternalInput")
    w4T = nc.dram_tensor("w4T", [64, 1], BF16, kind="ExternalInput")
    bredT = nc.dram_tensor("bredT", [P, KE], F32, kind="ExternalInput")
    binT = nc.dram_tensor("binT", [P, 2 * KE], F32, kind="ExternalInput")
    boutR = nc.dram_tensor("boutR", [1, E], BF16, kind="ExternalInput")
    b1T = nc.dram_tensor("b1T", [P, 2], F32, kind="ExternalInput")
    b2T = nc.dram_tensor("b2T", [P, 1], F32, kind="ExternalInput")
    b3T = nc.dram_tensor("b3T", [64, 1], F32, kind="ExternalInput")
    b4 = nc.dram_tensor("b4", [1, 1], F32, kind="ExternalInput")
    out = nc.dram_tensor("out", [BC, 1], F32, kind="ExternalOutput")

    from contextlib import ExitStack

    with tile.TileContext(nc) as tc, ExitStack() as ctx:
        const = ctx.enter_context(tc.tile_pool(name="const", bufs=1))
        bredT_sb = const.tile([P, KE], F32)
        nc.sync.dma_start(bredT_sb[:], bredT[:])
        binT_sb = const.tile([P, 2 * KE], F32)
        nc.sync.dma_start(binT_sb[:], binT[:])
        boutR_sb = const.tile([1, E], BF16)
        nc.sync.dma_start(boutR_sb[:], boutR[:])
        ones_sb = const.tile([1, BC], BF16)
        nc.any.memset(ones_sb[:], 1.0)
        b1T_sb = const.tile([P, 2], F32)
        nc.sync.dma_start(b1T_sb[:], b1T[:])
        b2T_sb = const.tile([P, 1], F32)
        nc.sync.dma_start(b2T_sb[:], b2T[:])
        b3T_sb = const.tile([64, 1], F32)
        nc.sync.dma_start(b3T_sb[:], b3T[:])
        b4_sb = const.tile([1, 1], F32)
        nc.sync.dma_start(b4_sb[:], b4[:])
        # indicator[p, j] = 1.0 if p // 32 == j else 0  (for z block-sums)
        ind_sb = const.tile([P, GS], BF16)
        nc.any.memset(ind_sb[:], 0.0)
        for j in range(GS):
            nc.any.memset(ind_sb[j * L:(j + 1) * L, j:j + 1], 1.0)
        ident_sb = const.tile([P, P], BF16)
        make_identity(nc, ident_sb)
        # block-diagonal mask: mask4[p, c] = 1.0 if c // 32 == p // 32
        mask4_sb = const.tile([P, P], BF16)
        nc.any.memset(mask4_sb[:], 0.0)
        for j in range(GS):
            nc.any.memset(mask4_sb[j * L:(j + 1) * L, j * L:(j + 1) * L], 1.0)

        # persistent activations (live across phases)
        acts = ctx.enter_context(tc.tile_pool(name="acts", bufs=1))
        hT_sb = acts.tile([P, KE * M], BF16)      # col = et*M + m
        u_sb = acts.tile([P, MT], F32)            # col = g (m-tile)
        zT_sb = acts.tile([P, KE * BC], BF16)     # col = ec*BC + s
        pT_sb = acts.tile([P, KE * BC], BF16)     # pooled.T, col = e2t*BC + s
        o1T_sb = acts.tile([P, 2 * BC], BF16)
        o2T_sb = acts.tile([P, BC], BF16)
        o3T_sb = acts.tile([64, BC], BF16)
        outT_sb = acts.tile([1, BC], F32)

        # ---------------- GEMM1: hT = relu(wredT.T-chain) ----------------
        # hT[e, m] = relu(sum_k wredT[k, e] * xT[k, m] + b_red[e])
        with ExitStack() as s1:
            xpool = s1.enter_context(tc.tile_pool(name="xT", bufs=1))
            xT_sb = xpool.tile([P, KX * M], BF16)  # col = kt*M + m
            nc.sync.dma_start(
                xT_sb[:].rearrange("p (kt m) -> p kt m", kt=KX),
                xT[:].rearrange("(kt p) m -> p kt m", p=P))
            wpool = s1.enter_context(tc.tile_pool(name="wred", bufs=2))
            ps1 = s1.enter_context(tc.tile_pool(name="ps1", bufs=4, space="PSUM"))
            EG = 2  # e-tiles per stripe
            for eg in range(KE // EG):
                stripe = wpool.tile([P, KX * EG * P], BF16)  # col = kt*(EG*P)+e_local
                nc.sync.dma_start(
                    stripe[:].rearrange("p (kt e) -> p kt e", kt=KX),
                    wredT[:, eg * EG * P:(eg + 1) * EG * P]
                    .rearrange("(kt p) e -> p kt e", p=P))
                for el in range(EG):
                    et = eg * EG + el
                    for mc in range(2):
                        psum = ps1.tile([P, 512], F32)
                        for kt in range(KX):
                            nc.tensor.matmul(
                                psum[:],
                                stripe[:, kt * EG * P + el * P:kt * EG * P + (el + 1) * P],
                                xT_sb[:, kt * M + mc * 512:kt * M + (mc + 1) * 512],
                                start=(kt == 0), stop=(kt == KX - 1))
                        nc.scalar.activation(
                            hT_sb[:, et * M + mc * 512:et * M + (mc + 1) * 512],
                            psum[:], AF.Relu, bias=bredT_sb[:, et:et + 1])

        # ---------------- GEMM2 + attention ----------------
        with ExitStack() as s2:
            vpool = s2.enter_context(tc.tile_pool(name="vctx", bufs=1))
            v_sb = vpool.tile([P, MT * E], BF16)    # col = mt*E + f
            ctx_sb = vpool.tile([P, MT * E], BF16)  # col = mt*E + e
            nc.any.memset(u_sb[:], 0.0)

            if bisect == 1:
                dpool = s2.enter_context(tc.tile_pool(name="dummy", bufs=1))
                t_ = dpool.tile([1, BC], F32)
                nc.vector.tensor_copy(t_[:], hT_sb[0:1, 0:BC])
                nc.sync.dma_start(out[:, 0:1], t_[0:1, :])
            # v = h @ w_in_v.T   (natural orientation; lhsT = hT)
            with ExitStack() as s2a:
              if bisect != 1:
                wvpool = s2a.enter_context(tc.tile_pool(name="winv", bufs=2))
                psv = s2a.enter_context(tc.tile_pool(name="psv", bufs=4, space="PSUM"))
                for fc in range(4):  # 512-wide f chunks of v
                    stripe = wvpool.tile([P, KE * 512], BF16)
                    nc.sync.dma_start(
                        stripe[:].rearrange("p (ke f) -> p ke f", ke=KE),
                        winT[:, 2 * E + fc * 512:2 * E + (fc + 1) * 512]
                        .rearrange("(ke p) f -> p ke f", p=P))
                    for mt in range(MT):
                        psum = psv.tile([P, 512], F32)
                        for ke in range(KE):
                            nc.tensor.matmul(
                                psum[:],
                                hT_sb[:, ke * M + mt * P:ke * M + (mt + 1) * P],
                                stripe[:, ke * 512:(ke + 1) * 512],
                                start=(ke == 0), stop=(ke == KE - 1))
                        nc.vector.tensor_copy(
                            v_sb[:, mt * E + fc * 512:mt * E + (fc + 1) * 512],
                            psum[:])

            # per-head: qT/kT GEMM then attention core
            with ExitStack() as s2b:
                wqk = s2b.enter_context(tc.tile_pool(name="winqk", bufs=2))
                qk_out = s2b.enter_context(tc.tile_pool(name="qkT", bufs=2))
                psqk = s2b.enter_context(tc.tile_pool(name="psqk", bufs=2, space="PSUM"))
                pss = s2b.enter_context(tc.tile_pool(name="pss", bufs=2, space="PSUM"))
                psc = s2b.enter_context(tc.tile_pool(name="psc", bufs=2, space="PSUM"))
                psu = s2b.enter_context(tc.tile_pool(name="psu", bufs=2, space="PSUM"))
                attp = s2b.enter_context(tc.tile_pool(name="attp", bufs=3))
                for h in range(NH if bisect != 1 else 0):
                    q_stripe = wqk.tile([P, KE * HD], BF16, tag="qs")
                    k_stripe = wqk.tile([P, KE * HD], BF16, tag="ks")
                    nc.sync.dma_start(
                        q_stripe[:].rearrange("p (ke f) -> p ke f", ke=KE),
                        winT[:, h * HD:(h + 1) * HD]
                        .rearrange("(ke p) f -> p ke f", p=P))
                    nc.sync.dma_start(
                        k_stripe[:].rearrange("p (ke f) -> p ke f", ke=KE),
                        winT[:, E + h * HD:E + (h + 1) * HD]
                        .rearrange("(ke p) f -> p ke f", p=P))
                    qT_sb = qk_out.tile([P, 2 * M], BF16, tag="qT")  # col = dc*M + m
                    kT_sb = qk_out.tile([P, 2 * M], BF16, tag="kT")
                    for dst, stripe, bcol0 in ((qT_sb, q_stripe, 2 * h),
                                               (kT_sb, k_stripe, KE + 2 * h)):
                        for dc in range(2):
                            for mc in range(2):
                                psum = psqk.tile([P, 512], F32)
                                for ke in range(KE):
                                    nc.tensor.matmul(
                                        psum[:],
                                        stripe[:, ke * HD + dc * P:ke * HD + (dc + 1) * P],
                                        hT_sb[:, ke * M + mc * 512:ke * M + (mc + 1) * 512],
                                        start=(ke == 0), stop=(ke == KE - 1))
                                nc.vector.tensor_scalar_add(
                                    dst[:, dc * M + mc * 512:dc * M + (mc + 1) * 512],
                                    psum[:], binT_sb[:, bcol0 + dc:bcol0 + dc + 1])

                    if bisect == 2:
                        continue
                    # attention core per sample-group g (4 samples / 128 rows).
                    # scores for the 4 samples computed as one [128,128] MM;
                    # off-diagonal 32x32 blocks are cross-sample garbage,
                    # masked out before use via mask4.
                    for g in range(MT):
                        ps_s = pss.tile([P, P], F32, tag="ps_s")
                        for dc in range(2):
                            nc.tensor.matmul(
                                ps_s[:],
                                qT_sb[:, dc * M + g * P:dc * M + (g + 1) * P],
                                kT_sb[:, dc * M + g * P:dc * M + (g + 1) * P],
                                start=(dc == 0), stop=(dc == 1))
                        exp4 = attp.tile([P, P], BF16, tag="exp")
                        nc.scalar.activation(exp4[:], ps_s[:], AF.Exp,
                                             scale=1.0 / np.sqrt(HD))
                        if bisect == 5:
                            nc.vector.tensor_copy(
                                ctx_sb[:, g * E + h * HD:g * E + h * HD + P],
                                exp4[:])
                            continue
                        # bd = block-diagonal masked exp, S = its row sums
                        bd = attp.tile([P, P], BF16, tag="bd")
                        S_col = attp.tile([P, 1], F32, tag="S")
                        nc.vector.tensor_tensor(out=bd[:], in0=exp4[:],
                                                in1=mask4_sb[:],
                                                op=mybir.AluOpType.mult)
                        nc.vector.tensor_reduce(S_col[:], bd[:],
                                                axis=mybir.AxisListType.X,
                                                op=mybir.AluOpType.add)
                        bdT_ps = pss.tile([P, P], BF16, tag="ps_s")
                        nc.tensor.transpose(bdT_ps[:], bd[:], ident_sb[:])
                        bdT = attp.tile([P, P], BF16, tag="bdT")
                        nc.vector.tensor_copy(bdT[:], bdT_ps[:])
                        if bisect == 8:
                            nc.vector.tensor_copy(
                                ctx_sb[:, g * E + h * HD:g * E + h * HD + P],
                                bdT[:])
                            continue
                        rS = attp.tile([P, 1], F32, tag="rS")
                        nc.vector.reciprocal(rS[:], S_col[:])
                        rS_bf = attp.tile([P, 1], BF16, tag="rSb")
                        nc.vector.tensor_copy(rS_bf[:], rS[:])
                        if bisect == 6:
                            nc.vector.tensor_copy(
                                ctx_sb[:, g * E + h * HD:g * E + h * HD + P],
                                bd[:])
                            nc.vector.tensor_copy(u_sb[:, g:g + 1], rS_bf[:])
                            continue
                        ps_ctx = psc.tile([P, HD], F32)
                        nc.tensor.matmul(
                            ps_ctx[:], bdT[:],
                            v_sb[:, g * E + h * HD:g * E + (h + 1) * HD],
                            start=True, stop=True)
                        nc.vector.tensor_scalar_mul(
                            ctx_sb[:, g * E + h * HD:g * E + (h + 1) * HD],
                            ps_ctx[:], rS[:])
                        if bisect == 7:
                            continue
                        ps_u = psu.tile([P, 1], F32)
                        nc.tensor.matmul(ps_u[:], bd[:], rS_bf[:],
                                         start=True, stop=True)
                        nc.vector.tensor_tensor(
                            out=u_sb[:, g:g + 1], in0=u_sb[:, g:g + 1],
                            in1=ps_u[:], op=mybir.AluOpType.add)

            if bisect == 2:
                dt_ = attp2 = None
                dpool = s2.enter_context(tc.tile_pool(name="dummy", bufs=1))
                t_ = dpool.tile([1, BC], F32)
                nc.vector.tensor_copy(t_[:], v_sb[0:1, 0:BC])
                nc.sync.dma_start(out[:, 0:1], t_[0:1, :])
            if bisect in (3, 5, 6, 7, 8):
                dpool = s2.enter_context(tc.tile_pool(name="dummy", bufs=1))
                t_ = dpool.tile([1, BC], F32)
                nc.vector.tensor_copy(t_[:], ctx_sb[0:1, 0:BC])
                nc.sync.dma_start(out[:, 0:1], t_[0:1, :])
            # ---------------- z = u @ ctx (per sample), transposed out ----------
            with ExitStack() as s3:
              if bisect not in (2, 3, 5, 6, 7, 8):
                zp = s3.enter_context(tc.tile_pool(name="zp", bufs=2))
                psz = s3.enter_context(tc.tile_pool(name="psz", bufs=1, space="PSUM"))
                ps_zT = psz.tile([P, KE * BC], F32)  # col = ec*BC + s  (512 f32)
                for g in range(MT):
                    tmp = zp.tile([P, E], BF16, tag="tmp")
                    # scale ctx rows by u (per-partition scalar), 1/(L*NH) folded
                    nc.vector.tensor_scalar(
                        out=tmp[:], in0=ctx_sb[:, g * E:(g + 1) * E],
                        scalar1=u_sb[:, g:g + 1], scalar2=1.0 / (L * NH),
                        op0=mybir.AluOpType.mult, op1=mybir.AluOpType.mult)
                    for ec in range(KE):
                        nc.tensor.matmul(
                            ps_zT[:, ec * BC + g * GS:ec * BC + (g + 1) * GS],
                            tmp[:, ec * P:(ec + 1) * P], ind_sb[:],
                            start=True, stop=True)
                nc.vector.tensor_copy(zT_sb[:], ps_zT[:])

        if bisect == 4:
            dpool = ctx.enter_context(tc.tile_pool(name="dummy", bufs=1))
            t_ = dpool.tile([1, BC], F32)
            nc.vector.tensor_copy(t_[:], zT_sb[0:1, 0:BC])
            nc.sync.dma_start(out[:, 0:1], t_[0:1, :])
        # ---------------- GEMM3: pooled = z @ w_out.T + b_out_eff ----------
        # natural orientation (M=32 partitions, N=512 free) so wout streams as
        # full contiguous rows; bias added as a rank-1 K=1 matmul; pooled is
        # then PE-transposed back to [E, 32] for the MLP chain.
        with ExitStack() as s4:
          if not bisect:
            wop = s4.enter_context(tc.tile_pool(name="wout", bufs=1))
            ps4 = s4.enter_context(tc.tile_pool(name="ps4", bufs=4, space="PSUM"))
            pstr = s4.enter_context(tc.tile_pool(name="pstr", bufs=2, space="PSUM"))
            ppool = s4.enter_context(tc.tile_pool(name="pooled", bufs=1))
            pooled_nat = ppool.tile([BC, E], BF16)
            wout_all = wop.tile([P, KE * E], BF16)  # col = ke*E + e2
            for ke in range(KE):
                nc.sync.dma_start(wout_all[:, ke * E:(ke + 1) * E],
                                  woutT[ke * P:(ke + 1) * P, :])
            for ec2 in range(4):
                psum = ps4.tile([BC, 512], F32, tag="mm")
                for ke in range(KE):
                    nc.tensor.matmul(
                        psum[:], zT_sb[:, ke * BC:(ke + 1) * BC],
                        wout_all[:, ke * E + ec2 * 512:ke * E + (ec2 + 1) * 512],
                        start=(ke == 0), stop=False)
                nc.tensor.matmul(psum[:], ones_sb[0:1, 0:BC],
                                 boutR_sb[0:1, ec2 * 512:(ec2 + 1) * 512],
                                 start=False, stop=True)
                nc.vector.tensor_copy(pooled_nat[:, ec2 * 512:(ec2 + 1) * 512],
                                      psum[:])
            # transpose pooled [32, E] -> pT [E-tiles of 128, 32]
            for ec in range(KE):
                trp = pstr.tile([P, BC], BF16, tag="tr")
                nc.tensor.transpose(trp[:], pooled_nat[:, ec * P:(ec + 1) * P],
                                    ident_sb[0:BC, 0:BC])
                nc.vector.tensor_copy(pT_sb[:, ec * BC:(ec + 1) * BC], trp[:])

            # ---------------- MLP head ----------------
            w1p = s4.enter_context(tc.tile_pool(name="w1p", bufs=2))
            for nt in range(2):
                stripe = w1p.tile([P, KE * P], BF16, tag="w1s")
                nc.sync.dma_start(
                    stripe[:].rearrange("p (ke n) -> p ke n", ke=KE),
                    w1T[:, nt * P:(nt + 1) * P]
                    .rearrange("(ke p) n -> p ke n", p=P))
                psum = ps4.tile([P, BC], F32, tag="mm")
                for ke in range(KE):
                    nc.tensor.matmul(psum[:], stripe[:, ke * P:(ke + 1) * P],
                                     pT_sb[:, ke * BC:(ke + 1) * BC],
                                     start=(ke == 0), stop=(ke == KE - 1))
                nc.scalar.activation(o1T_sb[:, nt * BC:(nt + 1) * BC], psum[:],
                                     AF.Relu, bias=b1T_sb[:, nt:nt + 1])
            w2_sb = w1p.tile([P, 2 * P], BF16, tag="w2s")
            nc.sync.dma_start(
                w2_sb[:].rearrange("p (ke n) -> p ke n", ke=2),
                w2T[:].rearrange("(ke p) n -> p ke n", p=P))
            psum = ps4.tile([P, BC], F32, tag="mm")
            for ke in range(2):
                nc.tensor.matmul(psum[:], w2_sb[:, ke * P:(ke + 1) * P],
                                 o1T_sb[:, ke * BC:(ke + 1) * BC],
                                 start=(ke == 0), stop=(ke == 1))
            nc.scalar.activation(o2T_sb[:], psum[:], AF.Relu,
                                 bias=b2T_sb[:, 0:1])
            w3_sb = w1p.tile([P, 64], BF16, tag="w3s")
            nc.sync.dma_start(w3_sb[:], w3T[:])
            psum3 = ps4.tile([P, BC], F32, tag="mm")
            nc.tensor.matmul(psum3[0:64, :], w3_sb[:], o2T_sb[:], start=True, stop=True)
            nc.scalar.activation(o3T_sb[:], psum3[0:64, :], AF.Relu, bias=b3T_sb[:, 0:1])
            w4_sb = w1p.tile([64, 1], BF16, tag="w4s")
            nc.sync.dma_start(w4_sb[:], w4T[:])
            psum4 = ps4.tile([P, BC], F32, tag="mm")
            nc.tensor.matmul(psum4[0:1, :], w4_sb[:], o3T_sb[:], start=True, stop=True)
            nc.scalar.activation(outT_sb[:], psum4[0:1, :], AF.Sigmoid,
                                 bias=b4_sb[:, 0:1])
            nc.sync.dma_start(out[:, 0:1], outT_sb[0:1, :])

    nc.compile()
    return nc


_BF = ml_dtypes.bfloat16


def _prep_shared(w_red, b_red, w_in, b_in, w_out, b_out, w1, b1, w2, b2, w3, b3,
                 w4, b4):
    f32 = np.float32
    w_red, w_in, w_out = (np.asarray(a, f32) for a in (w_red, w_in, w_out))
    b_in = np.asarray(b_in, f32)
    b_out_eff = np.asarray(b_out, f32) + w_out @ b_in[2 * E:3 * E]
    shared = {
        "wredT": w_red.T.astype(_BF),
        "winT": w_in.T.astype(_BF),
        "woutT": w_out.T.astype(_BF),
        "w1T": np.asarray(w1, f32).T.astype(_BF),
        "w2T": np.asarray(w2, f32).T.astype(_BF),
        "w3T": np.asarray(w3, f32).T.astype(_BF),
        "w4T": np.asarray(w4, f32).T.astype(_BF),
        "bredT": np.ascontiguousarray(np.asarray(b_red, f32).reshape(KE, P).T),
        "binT": np.ascontiguousarray(b_in[:2 * E].reshape(2 * KE, P).T),
        "boutR": b_out_eff.reshape(1, E).astype(_BF),
        "b1T": np.ascontiguousarray(np.asarray(b1, f32).reshape(2, P).T),
        "b2T": np.ascontiguousarray(np.asarray(b2, f32).reshape(1, P).T),
        "b3T": np.ascontiguousarray(np.asarray(b3, f32).reshape(1, 64).T),
        "b4": np.asarray(b4, f32).reshape(1, 1),
    }
    return shared


def kernel(x, w_red, b_red, w_in, b_in, w_out, b_out, w1, b1, w2, b2, w3, b3,
           w4, b4):
    global LAST_EXEC_TIME_NS
    x = np.asarray(x, np.float32)
    shared = _prep_shared(w_red, b_red, w_in, b_in, w_out, b_out, w1, b1, w2,
                          b2, w3, b3, w4, b4)
    in_maps = []
    for c in range(NCORES):
        xc = x[c * BC:(c + 1) * BC].reshape(M, H)
        in_maps.append({"xT": xc.T.astype(_BF), **shared})
    nc = _build_kernel()
    trace = os.environ.get("BASS_TRACE", "0") == "1"
    kw = {}
    if trace:
        _install_ntff_hook_shim()
        import concourse.bass_utils as _bu
        _bu.upload_artifacts = lambda d: str(d)  # no artifact bucket here
        tmpdir = os.environ.get("BASS_TRACE_DIR", "/tmp/bass_trace")
        os.makedirs(tmpdir, exist_ok=True)
        kw = {"trace": True, "tmpdir": tmpdir}
    res = run_bass_kernel_spmd(nc, in_maps, core_ids=list(range(NCORES)), **kw)
    LAST_EXEC_TIME_NS = res.exec_time_ns
    return np.concatenate([res.results[c]["out"] for c in range(NCORES)], axis=0)


if __name__ == "__main__":
    rng = np.random.default_rng(0)
    print("smoke test: building kernel only")
    _build_kernel()
    print("build OK")



# revision 6
# speedup vs baseline: 1.5921x; 1.5921x over previous
"""Trainium2 Bass kernel for AttentionMLPReduction (fp8 DoubleRow version).

Reference computation (per sample, B=256, L=32, H=4096, E=2048, NH=8, hd=256):
  h    = relu(x @ w_red.T + b_red)                  (B,L,E)
  qkv  = h @ w_in.T + b_in ; q,k,v = split(qkv)
  attn = softmax(q @ k.T / sqrt(hd))  per head      (B,NH,L,L)
  ctx  = attn @ v                                   (B,NH,L,hd) -> (B,L,E)
  attn_output = ctx @ w_out.T + b_out               (B,L,E)
  w_mean = attn.mean(heads); w_norm = w_mean / rowsum  (== w_mean)
  pooled = mean_q(w_norm @ attn_output)             (B,E)
  out = sigmoid(mlp(pooled))                        (B,1)

Algebraic simplifications:
  * w_norm == w_mean exactly (rows already sum to 1).
  * pooled[b] = u[b] @ attn_output[b] with u[b,l] = mean_q w_mean[b,q,l].
  * z[b] := u[b] @ ctx[b]; pooled = z @ w_out.T + b_out_eff with
    b_out_eff = b_out + w_out @ b_in_v (since sum_l u[b,l] == 1).
  * w_out is folded into the MLP head entirely:
      o1 = relu(pooled @ w1.T + b1) = relu(z @ (w1 @ w_out).T + b1_eff),
      b1_eff = b1 + w1 @ b_out_eff, so the (B,E)x(E,E) GEMM3 disappears.

Precision: the three big GEMMs (x@w_red, h@w_qk, h@w_v) run in fp8 e4m3 with
MatmulPerfMode.DoubleRow (2 k-rows per PE pass). Host pre-scales operands to
the e4m3 sweet range with power-of-2 factors (exact), and the dequant scales
fold into the psum->sbuf activation step:
  x*32, w_red*4096 -> h8 = 16*h via relu scale 1/8192
  h8 (=16*h), w_in*8192 -> q,k,v via scale 1/131072
Softmax block-diag masking is folded into the scores psum as a rank-5 matmul
adding -1600 to off-diagonal 32x32 blocks (exp(x/16-100) == 0 in bf16).

Sharding: pure data parallel over batch; 32 samples per core, weights
replicated (prepared/cast once on the host).

Per-core layouts (partition dim first):
  xT8   [H=4096, M=1024] fp8   (M = 32 samples x L=32), value = 32*x
  hT    [E=2048, M]      fp8   in SBUF, value = 16*h
  qT,kT per head-pair: [128, 4*M] bf16 (col = dtile*M + m), real values
  v     [M, E]           bf16   natural orientation
  ctx   [M, E]           bf16
  zT    [E, 32]          bf16 -> o1T [256,32] -> ... -> outT [1,32]
"""

import os
import numpy as np
import ml_dtypes

import concourse.bass as bass
import concourse.mybir as mybir
import concourse.tile as tile
from concourse import bacc
from concourse.bass_utils import run_bass_kernel_spmd
from concourse.masks import make_identity

BF16 = mybir.dt.bfloat16
F8 = mybir.dt.float8e4
F32 = mybir.dt.float32
AF = mybir.ActivationFunctionType
DR = mybir.MatmulPerfMode.DoubleRow

B, L, H, E, NH = 256, 32, 4096, 2048, 8
HD = E // NH  # 256
NCORES = 8
BC = B // NCORES  # 32 samples per core
M = BC * L  # 1024 rows per core
P = 128
KX = H // P  # 32 k-tiles for GEMM1
KE = E // P  # 16 k-tiles for E-contraction GEMMs
MT = M // P  # 8 m-tiles
GS = P // L  # 4 samples per partition-tile

# fp8 scaling (all powers of two -> exact to fold/unfold)
XS = 32.0        # x pre-scale
WRS = 4096.0     # w_red pre-scale
HS = 16.0        # h post-scale (stored h8 = HS*h)
WIS = 8192.0     # w_in pre-scale
G1_SCALE = HS / (XS * WRS)      # psum -> h8
QKV_SCALE = 1.0 / (HS * WIS)    # psum -> q/k/v

# module-level stash for the last run's HW exec time (ns), if traced
LAST_EXEC_TIME_NS = None


def _install_ntff_hook_shim():
    """antenv.axon_hooks is missing in this container; bass_utils imports it
    when trace=True under axon. Recreate it and register the ctypes-driven
    NRT profile hook from trn_boot if available."""
    import sys
    import types
    try:
        from antenv import axon_hooks  # noqa: F401
        return
    except ImportError:
        pass
    try:
        import antenv
    except ImportError:
        return
    m = types.ModuleType("antenv.axon_hooks")
    m._hook = None
    m.set_axon_ntff_profile_hook = lambda h: setattr(m, "_hook", h)
    m.get_axon_ntff_profile_hook = lambda: m._hook
    sys.modules["antenv.axon_hooks"] = m
    antenv.axon_hooks = m
    try:
        from trn_agent_boot.trn_boot import _ntff_profile_via_ctypes
        hook = _ntff_profile_via_ctypes("/opt/axon/libaxon_pjrt.so")
        if hook is not None:
            m._hook = hook
    except Exception:
        pass


def _build_kernel() -> bass.Bass:
    nc = bacc.Bacc(None, target_bir_lowering=False, debug=False)

    # ---- DRAM parameters (per-core shard views) ----
    xT8 = nc.dram_tensor("xT8", [H, M], F8, kind="ExternalInput")
    wredT8 = nc.dram_tensor("wredT8", [H, E], F8, kind="ExternalInput")
    winT8 = nc.dram_tensor("winT8", [E, 3 * E], F8, kind="ExternalInput")
    w1effT = nc.dram_tensor("w1effT", [E, 256], BF16, kind="ExternalInput")
    w2T = nc.dram_tensor("w2T", [256, P], BF16, kind="ExternalInput")
    w3T = nc.dram_tensor("w3T", [P, 64], BF16, kind="ExternalInput")
    w4T = nc.dram_tensor("w4T", [64, 1], BF16, kind="ExternalInput")
    bredT16 = nc.dram_tensor("bredT16", [P, KE], F32, kind="ExternalInput")
    binT = nc.dram_tensor("binT", [P, 2 * KE], F32, kind="ExternalInput")
    b1effT = nc.dram_tensor("b1effT", [P, 2], F32, kind="ExternalInput")
    b2T = nc.dram_tensor("b2T", [P, 1], F32, kind="ExternalInput")
    b3T = nc.dram_tensor("b3T", [64, 1], F32, kind="ExternalInput")
    b4 = nc.dram_tensor("b4", [1, 1], F32, kind="ExternalInput")
    mbl = nc.dram_tensor("mbl", [GS, P], BF16, kind="ExternalInput")
    mbr = nc.dram_tensor("mbr", [GS, P], BF16, kind="ExternalInput")
    out = nc.dram_tensor("out", [BC, 1], F32, kind="ExternalOutput")

    from contextlib import ExitStack

    with tile.TileContext(nc) as tc, ExitStack() as ctx:
        const = ctx.enter_context(tc.tile_pool(name="const", bufs=1))
        bredT_sb = const.tile([P, KE], F32)
        nc.sync.dma_start(bredT_sb[:], bredT16[:])
        binT_sb = const.tile([P, 2 * KE], F32)
        nc.sync.dma_start(binT_sb[:], binT[:])
        b1T_sb = const.tile([P, 2], F32)
        nc.sync.dma_start(b1T_sb[:], b1effT[:])
        b2T_sb = const.tile([P, 1], F32)
        nc.sync.dma_start(b2T_sb[:], b2T[:])
        b3T_sb = const.tile([64, 1], F32)
        nc.sync.dma_start(b3T_sb[:], b3T[:])
        b4_sb = const.tile([1, 1], F32)
        nc.sync.dma_start(b4_sb[:], b4[:])
        # indicator[p, j] = 1.0 if p // 32 == j else 0  (for z block-sums)
        ind_sb = const.tile([P, GS], BF16)
        nc.any.memset(ind_sb[:], 0.0)
        for j in range(GS):
            nc.any.memset(ind_sb[j * L:(j + 1) * L, j:j + 1], 1.0)
        ident_sb = const.tile([P, P], BF16)
        make_identity(nc, ident_sb)
        # rank-4 factors of the block-diag mask bias: -1600 off-diagonal,
        # 0 on-diagonal (scores psum += mbl.T @ mbr), host-prepared.
        mbl_sb = const.tile([GS, P], BF16)
        nc.sync.dma_start(mbl_sb[:], mbl[:])
        mbr_sb = const.tile([GS, P], BF16)
        nc.sync.dma_start(mbr_sb[:], mbr[:])

        # persistent activations (live across phases)
        acts = ctx.enter_context(tc.tile_pool(name="acts", bufs=1))
        hT_sb = acts.tile([P, KE * M], F8)         # col = et*M + m, = 16*h
        u_sb = acts.tile([P, MT], F32)             # col = g (m-tile)
        zT_sb = acts.tile([P, KE * BC], BF16)      # col = ec*BC + s
        o1T_sb = acts.tile([P, 2 * BC], BF16)
        o2T_sb = acts.tile([P, BC], BF16)
        o3T_sb = acts.tile([64, BC], BF16)
        outT_sb = acts.tile([1, BC], F32)

        # weights streamed early (scalar-engine DMA queue, overlaps GEMM1):
        # all 4 v-stripes + the folded w1 upfront; qk stripes double-buffered.
        wv = ctx.enter_context(tc.tile_pool(name="winv", bufs=1))
        v_stripes = []
        for fc in range(4):
            st = wv.tile([P, KE * 512], F8)
            nc.scalar.dma_start(
                st[:].rearrange("p (ke f) -> p ke f", ke=KE),
                winT8[:, 2 * E + fc * 512:2 * E + (fc + 1) * 512]
                .rearrange("(ke p) f -> p ke f", p=P))
            v_stripes.append(st)
        w1p = ctx.enter_context(tc.tile_pool(name="w1p", bufs=1))
        w1s = w1p.tile([P, KE * 256], BF16)
        nc.scalar.dma_start(
            w1s[:].rearrange("p (ke n) -> p ke n", ke=KE),
            w1effT[:].rearrange("(ke p) n -> p ke n", p=P))
        w2_sb = w1p.tile([P, 2 * P], BF16)
        nc.scalar.dma_start(
            w2_sb[:].rearrange("p (ke n) -> p ke n", ke=2),
            w2T[:].rearrange("(ke p) n -> p ke n", p=P))
        w3_sb = w1p.tile([P, 64], BF16)
        nc.scalar.dma_start(w3_sb[:], w3T[:])
        w4_sb = w1p.tile([64, 1], BF16)
        nc.scalar.dma_start(w4_sb[:], w4T[:])

        # ---------------- GEMM1: hT8 = relu-scale(wredT.T-chain) -----------
        # hT8[e, m] = 16*relu(sum_k wredT[k,e]*xT[k,m]/131072 + b_red[e])
        with ExitStack() as s1:
            xpool = s1.enter_context(tc.tile_pool(name="xT", bufs=1))
            xT_sb = xpool.tile([P, KX * M], F8)  # col = kt*M + m
            nc.sync.dma_start(
                xT_sb[:].rearrange("p (kt m) -> p kt m", kt=KX),
                xT8[:].rearrange("(kt p) m -> p kt m", p=P))
            x3 = xT_sb[:].rearrange("p (kt m) -> p kt m", kt=KX)
            wpool = s1.enter_context(tc.tile_pool(name="wred", bufs=2))
            ps1 = s1.enter_context(tc.tile_pool(name="ps1", bufs=2, space="PSUM"))
            EG = 4  # e-tiles per stripe
            for eg in range(KE // EG):
                stripe = wpool.tile([P, KX * EG * P], F8)  # col = kt*512+e_l
                nc.sync.dma_start(
                    stripe[:].rearrange("p (kt e) -> p kt e", kt=KX),
                    wredT8[:, eg * EG * P:(eg + 1) * EG * P]
                    .rearrange("(kt p) e -> p kt e", p=P))
                w3r = stripe[:].rearrange("p (kt e) -> p kt e", kt=KX)
                for el in range(EG):
                    et = eg * EG + el
                    psums = [ps1.tile([P, 512], F32, name=f"g1ps{i}", tag=f"g1ps{i}") for i in range(2)]
                    for kp in range(KX // 2):
                        for mc in range(2):
                            nc.tensor.matmul(
                                psums[mc][:],
                                w3r[:, 2 * kp:2 * kp + 2, el * P:(el + 1) * P],
                                x3[:, 2 * kp:2 * kp + 2, mc * 512:(mc + 1) * 512],
                                start=(kp == 0), stop=(kp == KX // 2 - 1),
                                perf_mode=DR)
                    for mc in range(2):
                        nc.scalar.activation(
                            hT_sb[:, et * M + mc * 512:et * M + (mc + 1) * 512],
                            psums[mc][:], AF.Relu, bias=bredT_sb[:, et:et + 1],
                            scale=G1_SCALE)
        h3 = hT_sb[:].rearrange("p (ke m) -> p ke m", ke=KE)

        # ---------------- GEMM2 (v) + per-head-pair qk + attention ---------
        with ExitStack() as s2:
            vpool = s2.enter_context(tc.tile_pool(name="vctx", bufs=1))
            v_sb = vpool.tile([P, MT * E], BF16)    # col = mt*E + f
            ctx_sb = vpool.tile([P, MT * E], BF16)  # col = mt*E + e
            nc.any.memset(u_sb[:], 0.0)

            # v = h @ w_in_v.T  (natural orientation; stationary = hT slices)
            with ExitStack() as s2a:
                psv = s2a.enter_context(tc.tile_pool(name="psv", bufs=1, space="PSUM"))
                for mt in range(MT):
                    psums = [psv.tile([P, 512], F32, name=f"vps{i}", tag=f"vps{i}") for i in range(4)]
                    for kp in range(KE // 2):
                        for fc in range(4):
                            st3 = v_stripes[fc][:].rearrange(
                                "p (ke f) -> p ke f", ke=KE)
                            nc.tensor.matmul(
                                psums[fc][:],
                                h3[:, 2 * kp:2 * kp + 2, mt * P:(mt + 1) * P],
                                st3[:, 2 * kp:2 * kp + 2, :],
                                start=(kp == 0), stop=(kp == KE // 2 - 1),
                                perf_mode=DR)
                    for fc in range(4):
                        nc.scalar.activation(
                            v_sb[:, mt * E + fc * 512:mt * E + (fc + 1) * 512],
                            psums[fc][:], AF.Copy, scale=QKV_SCALE)

            # per head-pair: q/k GEMM (fp8 DR), then attention core per head
            with ExitStack() as s2b:
                wqk = s2b.enter_context(tc.tile_pool(name="winqk", bufs=2))
                qk_out = s2b.enter_context(tc.tile_pool(name="qkT", bufs=2))
                psqk = s2b.enter_context(tc.tile_pool(name="psqk", bufs=2, space="PSUM"))
                pss = s2b.enter_context(tc.tile_pool(name="pss", bufs=2, space="PSUM"))
                psc = s2b.enter_context(tc.tile_pool(name="psc", bufs=1, space="PSUM"))
                psu = s2b.enter_context(tc.tile_pool(name="psu", bufs=1, space="PSUM"))
                attp = s2b.enter_context(tc.tile_pool(name="attp", bufs=3))
                for hp in range(NH // 2):  # head pair: heads 2hp, 2hp+1
                    q_stripe = wqk.tile([P, KE * 512], F8, tag="qs")
                    k_stripe = wqk.tile([P, KE * 512], F8, tag="ks")
                    nc.sync.dma_start(
                        q_stripe[:].rearrange("p (ke f) -> p ke f", ke=KE),
                        winT8[:, hp * 512:(hp + 1) * 512]
                        .rearrange("(ke p) f -> p ke f", p=P))
                    nc.sync.dma_start(
                        k_stripe[:].rearrange("p (ke f) -> p ke f", ke=KE),
                        winT8[:, E + hp * 512:E + (hp + 1) * 512]
                        .rearrange("(ke p) f -> p ke f", p=P))
                    # qT2/kT2: col = dl*M + m, dl in 0..3 (dtile = 4*hp + dl)
                    qT_sb = qk_out.tile([P, 4 * M], BF16, tag="qT")
                    kT_sb = qk_out.tile([P, 4 * M], BF16, tag="kT")
                    for dst, stripe, bcol0 in ((qT_sb, q_stripe, 4 * hp),
                                               (kT_sb, k_stripe, KE + 4 * hp)):
                        s3 = stripe[:].rearrange("p (ke f) -> p ke f", ke=KE)
                        for dl in range(4):
                            psums = [psqk.tile([P, 512], F32, name=f"qkps{i}", tag=f"qkps{i}") for i in range(2)]
                            for kp in range(KE // 2):
                                for mc in range(2):
                                    nc.tensor.matmul(
                                        psums[mc][:],
                                        s3[:, 2 * kp:2 * kp + 2, dl * P:(dl + 1) * P],
                                        h3[:, 2 * kp:2 * kp + 2, mc * 512:(mc + 1) * 512],
                                        start=(kp == 0), stop=(kp == KE // 2 - 1),
                                        perf_mode=DR)
                            for mc in range(2):
                                nc.scalar.activation(
                                    dst[:, dl * M + mc * 512:dl * M + (mc + 1) * 512],
                                    psums[mc][:], AF.Identity,
                                    bias=binT_sb[:, bcol0 + dl:bcol0 + dl + 1],
                                    scale=QKV_SCALE)

                    for hh in range(2):
                        h = 2 * hp + hh
                        d0 = 2 * hh  # dl of this head's first d-tile
                        # attention core per sample-group g (4 samples).
                        # scores for 4 samples as one [128,128] MM; cross-
                        # sample blocks are killed by the -1600 psum bias.
                        for g in range(MT):
                            ps_s = pss.tile([P, P], F32, tag="ps_s")
                            for dc in range(2):
                                nc.tensor.matmul(
                                    ps_s[:],
                                    qT_sb[:, (d0 + dc) * M + g * P:(d0 + dc) * M + (g + 1) * P],
                                    kT_sb[:, (d0 + dc) * M + g * P:(d0 + dc) * M + (g + 1) * P],
                                    start=(dc == 0), stop=False)
                            nc.tensor.matmul(ps_s[:], mbl_sb[:], mbr_sb[:],
                                             start=False, stop=True)
                            # bd = block-diag masked exp(scores/sqrt(hd))
                            bd = attp.tile([P, P], BF16, tag="bd")
                            nc.scalar.activation(bd[:], ps_s[:], AF.Exp,
                                                 scale=1.0 / np.sqrt(HD))
                            S_col = attp.tile([P, 1], F32, tag="S")
                            nc.vector.tensor_reduce(S_col[:], bd[:],
                                                    axis=mybir.AxisListType.X,
                                                    op=mybir.AluOpType.add)
                            bdT_ps = pss.tile([P, P], BF16, tag="ps_s")
                            nc.tensor.transpose(bdT_ps[:], bd[:], ident_sb[:])
                            bdT = attp.tile([P, P], BF16, tag="bdT")
                            nc.vector.tensor_copy(bdT[:], bdT_ps[:])
                            rS = attp.tile([P, 1], F32, tag="rS")
                            nc.vector.reciprocal(rS[:], S_col[:])
                            rS_bf = attp.tile([P, 1], BF16, tag="rSb")
                            nc.vector.tensor_copy(rS_bf[:], rS[:])
                            ps_ctx = psc.tile([P, HD], F32)
                            nc.tensor.matmul(
                                ps_ctx[:], bdT[:],
                                v_sb[:, g * E + h * HD:g * E + (h + 1) * HD],
                                start=True, stop=True)
                            nc.scalar.activation(
                                ctx_sb[:, g * E + h * HD:g * E + (h + 1) * HD],
                                ps_ctx[:], AF.Copy, scale=rS[:, 0:1])
                            ps_u = psu.tile([P, 1], F32)
                            nc.tensor.matmul(ps_u[:], bd[:], rS_bf[:],
                                             start=True, stop=True)
                            nc.vector.tensor_tensor(
                                out=u_sb[:, g:g + 1], in0=u_sb[:, g:g + 1],
                                in1=ps_u[:], op=mybir.AluOpType.add)

            # ---------------- z = u @ ctx (per sample), transposed out -----
            # zT[e, s] = sum_p ctx[p, e] * (ind[p, s]*u[p]/256): scale the
            # tiny indicator instead of the [128, 2048] ctx rows.
            with ExitStack() as s3:
                zp = s3.enter_context(tc.tile_pool(name="zp", bufs=2))
                psz = s3.enter_context(tc.tile_pool(name="psz", bufs=1, space="PSUM"))
                ps_zT = psz.tile([P, KE * BC], F32)  # col = ec*BC + s
                for g in range(MT):
                    ind_u = zp.tile([P, GS], BF16, tag="iu")
                    nc.vector.tensor_scalar(
                        out=ind_u[:], in0=ind_sb[:],
                        scalar1=u_sb[:, g:g + 1], scalar2=1.0 / (L * NH),
                        op0=mybir.AluOpType.mult, op1=mybir.AluOpType.mult)
                    for ec in range(KE):
                        nc.tensor.matmul(
                            ps_zT[:, ec * BC + g * GS:ec * BC + (g + 1) * GS],
                            ctx_sb[:, g * E + ec * P:g * E + (ec + 1) * P],
                            ind_u[:], start=True, stop=True)
                nc.vector.tensor_copy(zT_sb[:], ps_zT[:])

        # ---------------- MLP head (w_out folded into w1eff) ----------------
        with ExitStack() as s4:
            ps4 = s4.enter_context(tc.tile_pool(name="ps4", bufs=4, space="PSUM"))
            w1s3 = w1s[:].rearrange("p (ke n) -> p ke n", ke=KE)
            for nt in range(2):
                psum = ps4.tile([P, BC], F32, tag="mm")
                for ke in range(KE):
                    nc.tensor.matmul(psum[:], w1s3[:, ke, nt * P:(nt + 1) * P],
                                     zT_sb[:, ke * BC:(ke + 1) * BC],
                                     start=(ke == 0), stop=(ke == KE - 1))
                nc.scalar.activation(o1T_sb[:, nt * BC:(nt + 1) * BC], psum[:],
                                     AF.Relu, bias=b1T_sb[:, nt:nt + 1])
            psum = ps4.tile([P, BC], F32, tag="mm")
            w2s3 = w2_sb[:].rearrange("p (ke n) -> p ke n", ke=2)
            for ke in range(2):
                nc.tensor.matmul(psum[:], w2s3[:, ke, :],
                                 o1T_sb[:, ke * BC:(ke + 1) * BC],
                                 start=(ke == 0), stop=(ke == 1))
            nc.scalar.activation(o2T_sb[:], psum[:], AF.Relu,
                                 bias=b2T_sb[:, 0:1])
            psum3 = ps4.tile([P, BC], F32, tag="mm")
            nc.tensor.matmul(psum3[0:64, :], w3_sb[:], o2T_sb[:], start=True, stop=True)
            nc.scalar.activation(o3T_sb[:], psum3[0:64, :], AF.Relu, bias=b3T_sb[:, 0:1])
            psum4 = ps4.tile([P, BC], F32, tag="mm")
            nc.tensor.matmul(psum4[0:1, :], w4_sb[:], o3T_sb[:], start=True, stop=True)
            nc.scalar.activation(outT_sb[:], psum4[0:1, :], AF.Sigmoid,
                                 bias=b4_sb[:, 0:1])
            nc.sync.dma_start(out[:, 0:1], outT_sb[0:1, :])

    nc.compile()
    return nc


_BF = ml_dtypes.bfloat16
_F8 = ml_dtypes.float8_e4m3


def _q8(a, scale):
    return np.clip(np.asarray(a, np.float32) * scale, -240.0, 240.0).astype(_F8)


def _mbl():
    m = np.zeros((GS, P), np.float32)
    for j in range(GS):
        m[j, j * L:(j + 1) * L] = 1.0
    return m.astype(_BF)


def _mbr():
    m = np.full((GS, P), -1600.0, np.float32)
    for j in range(GS):
        m[j, j * L:(j + 1) * L] = 0.0
    return m.astype(_BF)


def _prep_shared(w_red, b_red, w_in, b_in, w_out, b_out, w1, b1, w2, b2, w3, b3,
                 w4, b4):
    f32 = np.float32
    w_red, w_in, w_out = (np.asarray(a, f32) for a in (w_red, w_in, w_out))
    w1 = np.asarray(w1, f32)
    b_in = np.asarray(b_in, f32)
    b_out_eff = np.asarray(b_out, f32) + w_out @ b_in[2 * E:3 * E]
    w1_eff = w1 @ w_out                     # (256, E)
    b1_eff = np.asarray(b1, f32) + w1 @ b_out_eff
    shared = {
        "wredT8": _q8(w_red.T, WRS),
        "winT8": _q8(w_in.T, WIS),
        "w1effT": np.ascontiguousarray(w1_eff.T).astype(_BF),
        "w2T": np.ascontiguousarray(np.asarray(w2, f32).T).astype(_BF),
        "w3T": np.ascontiguousarray(np.asarray(w3, f32).T).astype(_BF),
        "w4T": np.ascontiguousarray(np.asarray(w4, f32).T).astype(_BF),
        "bredT16": np.ascontiguousarray(
            (np.asarray(b_red, f32) * HS).reshape(KE, P).T),
        "binT": np.ascontiguousarray(b_in[:2 * E].reshape(2 * KE, P).T),
        "b1effT": np.ascontiguousarray(b1_eff.reshape(2, P).T),
        "b2T": np.ascontiguousarray(np.asarray(b2, f32).reshape(1, P).T),
        "b3T": np.ascontiguousarray(np.asarray(b3, f32).reshape(1, 64).T),
        "b4": np.asarray(b4, f32).reshape(1, 1),
        "mbl": _mbl(), "mbr": _mbr(),
    }
    return shared


def kernel(x, w_red, b_red, w_in, b_in, w_out, b_out, w1, b1, w2, b2, w3, b3,
           w4, b4):
    global LAST_EXEC_TIME_NS
    x = np.asarray(x, np.float32)
    shared = _prep_shared(w_red, b_red, w_in, b_in, w_out, b_out, w1, b1, w2,
                          b2, w3, b3, w4, b4)
    in_maps = []
    for c in range(NCORES):
        xc = x[c * BC:(c + 1) * BC].reshape(M, H)
        in_maps.append({"xT8": _q8(xc.T, XS), **shared})
    nc = _build_kernel()
    trace = os.environ.get("BASS_TRACE", "0") == "1"
    kw = {}
    if trace:
        _install_ntff_hook_shim()
        import concourse.bass_utils as _bu
        _bu.upload_artifacts = lambda d: str(d)  # no artifact bucket here
        tmpdir = os.environ.get("BASS_TRACE_DIR", "/tmp/bass_trace")
        os.makedirs(tmpdir, exist_ok=True)
        kw = {"trace": True, "tmpdir": tmpdir}
    res = run_bass_kernel_spmd(nc, in_maps, core_ids=list(range(NCORES)), **kw)
    LAST_EXEC_TIME_NS = res.exec_time_ns
    return np.concatenate([res.results[c]["out"] for c in range(NCORES)], axis=0)


if __name__ == "__main__":
    print("smoke test: building kernel only")
    _build_kernel()
    print("build OK")


# revision 8
# speedup vs baseline: 1.6630x; 1.0446x over previous
"""Trainium2 Bass kernel for AttentionMLPReduction (fp8 DoubleRow version).

Reference computation (per sample, B=256, L=32, H=4096, E=2048, NH=8, hd=256):
  h    = relu(x @ w_red.T + b_red)                  (B,L,E)
  qkv  = h @ w_in.T + b_in ; q,k,v = split(qkv)
  attn = softmax(q @ k.T / sqrt(hd))  per head      (B,NH,L,L)
  ctx  = attn @ v                                   (B,NH,L,hd) -> (B,L,E)
  attn_output = ctx @ w_out.T + b_out               (B,L,E)
  w_mean = attn.mean(heads); w_norm = w_mean / rowsum  (== w_mean)
  pooled = mean_q(w_norm @ attn_output)             (B,E)
  out = sigmoid(mlp(pooled))                        (B,1)

Algebraic simplifications:
  * w_norm == w_mean exactly (rows already sum to 1).
  * pooled[b] = u[b] @ attn_output[b] with u[b,l] = mean_q w_mean[b,q,l].
  * z[b] := u[b] @ ctx[b]; pooled = z @ w_out.T + b_out_eff with
    b_out_eff = b_out + w_out @ b_in_v (since sum_l u[b,l] == 1).
  * w_out is folded into the MLP head entirely:
      o1 = relu(pooled @ w1.T + b1) = relu(z @ (w1 @ w_out).T + b1_eff),
      b1_eff = b1 + w1 @ b_out_eff, so the (B,E)x(E,E) GEMM3 disappears.

Precision: the three big GEMMs (x@w_red, h@w_qk, h@w_v) run in fp8 e4m3 with
MatmulPerfMode.DoubleRow (2 k-rows per PE pass). Host pre-scales operands to
the e4m3 sweet range with power-of-2 factors (exact), and the dequant scales
fold into the psum->sbuf activation step:
  x*32, w_red*4096 -> h8 = 16*h via relu scale 1/8192
  h8 (=16*h), w_in*8192 -> q,k,v via scale 1/131072
Softmax block-diag masking is folded into the scores psum as a rank-5 matmul
adding -1600 to off-diagonal 32x32 blocks (exp(x/16-100) == 0 in bf16).

Sharding: pure data parallel over batch; 32 samples per core, weights
replicated (prepared/cast once on the host).

Per-core layouts (partition dim first):
  xT8   [H=4096, M=1024] fp8   (M = 32 samples x L=32), value = 32*x
  hT    [E=2048, M]      fp8   in SBUF, value = 16*h
  qT,kT per head-pair: [128, 4*M] bf16 (col = dtile*M + m), real values
  v     [M, E]           bf16   natural orientation
  ctx   [M, E]           bf16
  zT    [E, 32]          bf16 -> o1T [256,32] -> ... -> outT [1,32]
"""

import os
import numpy as np
import ml_dtypes

import concourse.bass as bass
import concourse.mybir as mybir
import concourse.tile as tile
from concourse import bacc
from concourse.bass_utils import run_bass_kernel_spmd
from concourse.masks import make_identity

BF16 = mybir.dt.bfloat16
F8 = mybir.dt.float8e4
F32 = mybir.dt.float32
AF = mybir.ActivationFunctionType
DR = mybir.MatmulPerfMode.DoubleRow

B, L, H, E, NH = 256, 32, 4096, 2048, 8
HD = E // NH  # 256
NCORES = 8
BC = B // NCORES  # 32 samples per core
M = BC * L  # 1024 rows per core
P = 128
KX = H // P  # 32 k-tiles for GEMM1
KE = E // P  # 16 k-tiles for E-contraction GEMMs
MT = M // P  # 8 m-tiles
GS = P // L  # 4 samples per partition-tile

# fp8 scaling (all powers of two -> exact to fold/unfold)
XS = 32.0        # x pre-scale
WRS = 4096.0     # w_red pre-scale
HS = 16.0        # h post-scale (stored h8 = HS*h)
WIS = 8192.0     # w_in pre-scale
G1_SCALE = HS / (XS * WRS)      # psum -> h8
QKV_SCALE = 1.0 / (HS * WIS)    # psum -> q/k/v

# module-level stash for the last run's HW exec time (ns), if traced
LAST_EXEC_TIME_NS = None


def _install_ntff_hook_shim():
    """antenv.axon_hooks is missing in this container; bass_utils imports it
    when trace=True under axon. Recreate it and register the ctypes-driven
    NRT profile hook from trn_boot if available."""
    import sys
    import types
    try:
        from antenv import axon_hooks  # noqa: F401
        return
    except ImportError:
        pass
    try:
        import antenv
    except ImportError:
        return
    m = types.ModuleType("antenv.axon_hooks")
    m._hook = None
    m.set_axon_ntff_profile_hook = lambda h: setattr(m, "_hook", h)
    m.get_axon_ntff_profile_hook = lambda: m._hook
    sys.modules["antenv.axon_hooks"] = m
    antenv.axon_hooks = m
    try:
        from trn_agent_boot.trn_boot import _ntff_profile_via_ctypes
        hook = _ntff_profile_via_ctypes("/opt/axon/libaxon_pjrt.so")
        if hook is not None:
            m._hook = hook
    except Exception:
        pass


def _build_kernel() -> bass.Bass:
    nc = bacc.Bacc(None, target_bir_lowering=False, debug=False)

    # ---- DRAM parameters (per-core shard views) ----
    xT8 = nc.dram_tensor("xT8", [H, M], F8, kind="ExternalInput")
    wredT8 = nc.dram_tensor("wredT8", [H, E], F8, kind="ExternalInput")
    winT8 = nc.dram_tensor("winT8", [E, 3 * E], F8, kind="ExternalInput")
    w1effT = nc.dram_tensor("w1effT", [E, 256], BF16, kind="ExternalInput")
    w2T = nc.dram_tensor("w2T", [256, P], BF16, kind="ExternalInput")
    w3T = nc.dram_tensor("w3T", [P, 64], BF16, kind="ExternalInput")
    w4T = nc.dram_tensor("w4T", [64, 1], BF16, kind="ExternalInput")
    bredT16 = nc.dram_tensor("bredT16", [P, KE], F32, kind="ExternalInput")
    binT = nc.dram_tensor("binT", [P, 2 * KE], F32, kind="ExternalInput")
    b1effT = nc.dram_tensor("b1effT", [P, 2], F32, kind="ExternalInput")
    b2T = nc.dram_tensor("b2T", [P, 1], F32, kind="ExternalInput")
    b3T = nc.dram_tensor("b3T", [64, 1], F32, kind="ExternalInput")
    b4 = nc.dram_tensor("b4", [1, 1], F32, kind="ExternalInput")
    mbl = nc.dram_tensor("mbl", [GS, P], BF16, kind="ExternalInput")
    mbr = nc.dram_tensor("mbr", [GS, P], BF16, kind="ExternalInput")
    out = nc.dram_tensor("out", [BC, 1], F32, kind="ExternalOutput")

    from contextlib import ExitStack

    with tile.TileContext(nc) as tc, ExitStack() as ctx:
        const = ctx.enter_context(tc.tile_pool(name="const", bufs=1))
        bredT_sb = const.tile([P, KE], F32)
        nc.sync.dma_start(bredT_sb[:], bredT16[:])
        binT_sb = const.tile([P, 2 * KE], F32)
        nc.sync.dma_start(binT_sb[:], binT[:])
        b1T_sb = const.tile([P, 2], F32)
        nc.sync.dma_start(b1T_sb[:], b1effT[:])
        b2T_sb = const.tile([P, 1], F32)
        nc.sync.dma_start(b2T_sb[:], b2T[:])
        b3T_sb = const.tile([64, 1], F32)
        nc.sync.dma_start(b3T_sb[:], b3T[:])
        b4_sb = const.tile([1, 1], F32)
        nc.sync.dma_start(b4_sb[:], b4[:])
        # indicator[p, j] = 1.0 if p // 32 == j else 0  (for z block-sums)
        ind_sb = const.tile([P, GS], BF16)
        nc.any.memset(ind_sb[:], 0.0)
        for j in range(GS):
            nc.any.memset(ind_sb[j * L:(j + 1) * L, j:j + 1], 1.0)
        ident_sb = const.tile([P, P], BF16)
        make_identity(nc, ident_sb)
        # rank-4 factors of the block-diag mask bias: -1600 off-diagonal,
        # 0 on-diagonal (scores psum += mbl.T @ mbr), host-prepared.
        mbl_sb = const.tile([GS, P], BF16)
        nc.sync.dma_start(mbl_sb[:], mbl[:])
        mbr_sb = const.tile([GS, P], BF16)
        nc.sync.dma_start(mbr_sb[:], mbr[:])

        # persistent activations (live across phases)
        acts = ctx.enter_context(tc.tile_pool(name="acts", bufs=1))
        hT_sb = acts.tile([P, KE * M], F8)         # col = et*M + m, = 16*h
        u_sb = acts.tile([P, MT], F32)             # col = g (m-tile)
        zT_sb = acts.tile([P, KE * BC], BF16)      # col = ec*BC + s
        o1T_sb = acts.tile([P, 2 * BC], BF16)
        o2T_sb = acts.tile([P, BC], BF16)
        o3T_sb = acts.tile([64, BC], BF16)
        outT_sb = acts.tile([1, BC], F32)

        # weights streamed early (scalar-engine DMA queue, overlaps GEMM1):
        # all 4 v-stripes + the folded w1 upfront; qk stripes double-buffered.
        wv = ctx.enter_context(tc.tile_pool(name="winv", bufs=1))
        v_stripes = []
        for fc in range(4):
            st = wv.tile([P, KE * 512], F8)
            nc.sync.dma_start(
                st[:].rearrange("p (ke f) -> p ke f", ke=KE),
                winT8[:, 2 * E + fc * 512:2 * E + (fc + 1) * 512]
                .rearrange("(ke p) f -> p ke f", p=P))
            v_stripes.append(st)
        w1p = ctx.enter_context(tc.tile_pool(name="w1p", bufs=1))
        w1s = w1p.tile([P, KE * 256], BF16)
        nc.sync.dma_start(
            w1s[:].rearrange("p (ke n) -> p ke n", ke=KE),
            w1effT[:].rearrange("(ke p) n -> p ke n", p=P))
        w2_sb = w1p.tile([P, 2 * P], BF16)
        nc.sync.dma_start(
            w2_sb[:].rearrange("p (ke n) -> p ke n", ke=2),
            w2T[:].rearrange("(ke p) n -> p ke n", p=P))
        w3_sb = w1p.tile([P, 64], BF16)
        nc.sync.dma_start(w3_sb[:], w3T[:])
        w4_sb = w1p.tile([64, 1], BF16)
        nc.sync.dma_start(w4_sb[:], w4T[:])

        # ---------------- GEMM1: hT8 = relu-scale(wredT.T-chain) -----------
        # hT8[e, m] = 16*relu(sum_k wredT[k,e]*xT[k,m]/131072 + b_red[e])
        with ExitStack() as s1:
            xpool = s1.enter_context(tc.tile_pool(name="xT", bufs=1))
            xT_sb = xpool.tile([P, KX * M], F8)  # col = kt*M + m
            nc.scalar.dma_start(
                xT_sb[:].rearrange("p (kt m) -> p kt m", kt=KX),
                xT8[:].rearrange("(kt p) m -> p kt m", p=P))
            x3 = xT_sb[:].rearrange("p (kt m) -> p kt m", kt=KX)
            wpool = s1.enter_context(tc.tile_pool(name="wred", bufs=2))
            ps1 = s1.enter_context(tc.tile_pool(name="ps1", bufs=2, space="PSUM"))
            EG = 4  # e-tiles per stripe
            for eg in range(KE // EG):
                stripe = wpool.tile([P, KX * EG * P], F8)  # col = kt*512+e_l
                nc.sync.dma_start(
                    stripe[:].rearrange("p (kt e) -> p kt e", kt=KX),
                    wredT8[:, eg * EG * P:(eg + 1) * EG * P]
                    .rearrange("(kt p) e -> p kt e", p=P))
                w3r = stripe[:].rearrange("p (kt e) -> p kt e", kt=KX)
                for el in range(EG):
                    et = eg * EG + el
                    psums = [ps1.tile([P, 512], F32, name=f"g1ps{i}", tag=f"g1ps{i}") for i in range(2)]
                    for kp in range(KX // 2):
                        for mc in range(2):
                            nc.tensor.matmul(
                                psums[mc][:],
                                w3r[:, 2 * kp:2 * kp + 2, el * P:(el + 1) * P],
                                x3[:, 2 * kp:2 * kp + 2, mc * 512:(mc + 1) * 512],
                                start=(kp == 0), stop=(kp == KX // 2 - 1),
                                perf_mode=DR)
                    for mc in range(2):
                        nc.scalar.activation(
                            hT_sb[:, et * M + mc * 512:et * M + (mc + 1) * 512],
                            psums[mc][:], AF.Relu, bias=bredT_sb[:, et:et + 1],
                            scale=G1_SCALE)
        h3 = hT_sb[:].rearrange("p (ke m) -> p ke m", ke=KE)

        # ---------------- GEMM2 (v) + per-head-pair qk + attention ---------
        with ExitStack() as s2:
            vpool = s2.enter_context(tc.tile_pool(name="vctx", bufs=1))
            v_sb = vpool.tile([P, MT * E], BF16)    # col = mt*E + f
            ctx_sb = vpool.tile([P, MT * E], BF16)  # col = mt*E + e
            nc.any.memset(u_sb[:], 0.0)

            # v = h @ w_in_v.T  (natural orientation; stationary = hT slices)
            with ExitStack() as s2a:
                psv = s2a.enter_context(tc.tile_pool(name="psv", bufs=2, space="PSUM"))
                for mt in range(MT):
                    psums = [psv.tile([P, 512], F32, name=f"vps{i}", tag=f"vps{i}") for i in range(4)]
                    for kp in range(KE // 2):
                        for fc in range(4):
                            st3 = v_stripes[fc][:].rearrange(
                                "p (ke f) -> p ke f", ke=KE)
                            nc.tensor.matmul(
                                psums[fc][:],
                                h3[:, 2 * kp:2 * kp + 2, mt * P:(mt + 1) * P],
                                st3[:, 2 * kp:2 * kp + 2, :],
                                start=(kp == 0), stop=(kp == KE // 2 - 1),
                                perf_mode=DR)
                    for fc in range(4):
                        nc.scalar.activation(
                            v_sb[:, mt * E + fc * 512:mt * E + (fc + 1) * 512],
                            psums[fc][:], AF.Copy, scale=QKV_SCALE)

            # per head-pair: q/k GEMM (fp8 DR), then attention core per head
            with ExitStack() as s2b:
                wqk = s2b.enter_context(tc.tile_pool(name="winqk", bufs=2))
                qk_out = s2b.enter_context(tc.tile_pool(name="qkT", bufs=2))
                psqk = s2b.enter_context(tc.tile_pool(name="psqk", bufs=2, space="PSUM"))
                pss = s2b.enter_context(tc.tile_pool(name="pss", bufs=2, space="PSUM"))
                psc = s2b.enter_context(tc.tile_pool(name="psc", bufs=1, space="PSUM"))
                psu = s2b.enter_context(tc.tile_pool(name="psu", bufs=1, space="PSUM"))
                attp = s2b.enter_context(tc.tile_pool(name="attp", bufs=3))
                ps_zT = None  # allocated from psqk during the last pair
                for hp in range(NH // 2):  # head pair: heads 2hp, 2hp+1
                    q_stripe = wqk.tile([P, KE * 512], F8, tag="qs")
                    k_stripe = wqk.tile([P, KE * 512], F8, tag="ks")
                    nc.sync.dma_start(
                        q_stripe[:].rearrange("p (ke f) -> p ke f", ke=KE),
                        winT8[:, hp * 512:(hp + 1) * 512]
                        .rearrange("(ke p) f -> p ke f", p=P))
                    nc.sync.dma_start(
                        k_stripe[:].rearrange("p (ke f) -> p ke f", ke=KE),
                        winT8[:, E + hp * 512:E + (hp + 1) * 512]
                        .rearrange("(ke p) f -> p ke f", p=P))
                    # qT2/kT2: col = dl*M + m, dl in 0..3 (dtile = 4*hp + dl)
                    qT_sb = qk_out.tile([P, 4 * M], BF16, tag="qT")
                    kT_sb = qk_out.tile([P, 4 * M], BF16, tag="kT")
                    for dst, stripe, bcol0 in ((qT_sb, q_stripe, 4 * hp),
                                               (kT_sb, k_stripe, KE + 4 * hp)):
                        s3 = stripe[:].rearrange("p (ke f) -> p ke f", ke=KE)
                        for dl in range(4):
                            psums = [psqk.tile([P, 512], F32, name=f"qkps{i}", tag=f"qkps{i}") for i in range(2)]
                            for kp in range(KE // 2):
                                for mc in range(2):
                                    nc.tensor.matmul(
                                        psums[mc][:],
                                        s3[:, 2 * kp:2 * kp + 2, dl * P:(dl + 1) * P],
                                        h3[:, 2 * kp:2 * kp + 2, mc * 512:(mc + 1) * 512],
                                        start=(kp == 0), stop=(kp == KE // 2 - 1),
                                        perf_mode=DR)
                            for mc in range(2):
                                nc.scalar.activation(
                                    dst[:, dl * M + mc * 512:dl * M + (mc + 1) * 512],
                                    psums[mc][:], AF.Identity,
                                    bias=binT_sb[:, bcol0 + dl:bcol0 + dl + 1],
                                    scale=QKV_SCALE)

                    if hp == NH // 2 - 1:
                        # qk psum banks are idle during attention; reuse one
                        # for the z accumulator (z fused into head 7's loop).
                        ps_zT = psqk.tile([P, KE * BC], F32, name="ps_zT",
                                          tag="qkps0")
                    for hh in range(2):
                        h = 2 * hp + hh
                        d0 = 2 * hh  # dl of this head's first d-tile
                        # attention core per sample-group g (4 samples).
                        # scores for 4 samples as one [128,128] MM; cross-
                        # sample blocks are killed by the -1600 psum bias.
                        for g in range(MT):
                            ps_s = pss.tile([P, P], F32, tag="ps_s")
                            for dc in range(2):
                                nc.tensor.matmul(
                                    ps_s[:],
                                    qT_sb[:, (d0 + dc) * M + g * P:(d0 + dc) * M + (g + 1) * P],
                                    kT_sb[:, (d0 + dc) * M + g * P:(d0 + dc) * M + (g + 1) * P],
                                    start=(dc == 0), stop=False)
                            nc.tensor.matmul(ps_s[:], mbl_sb[:], mbr_sb[:],
                                             start=False, stop=True)
                            # bd = block-diag masked exp(scores/sqrt(hd))
                            bd = attp.tile([P, P], BF16, tag="bd")
                            nc.scalar.activation(bd[:], ps_s[:], AF.Exp,
                                                 scale=1.0 / np.sqrt(HD))
                            S_col = attp.tile([P, 1], F32, tag="S")
                            nc.vector.tensor_reduce(S_col[:], bd[:],
                                                    axis=mybir.AxisListType.X,
                                                    op=mybir.AluOpType.add)
                            bdT_ps = pss.tile([P, P], BF16, tag="ps_s")
                            nc.tensor.transpose(bdT_ps[:], bd[:], ident_sb[:])
                            bdT = attp.tile([P, P], BF16, tag="bdT")
                            nc.vector.tensor_copy(bdT[:], bdT_ps[:])
                            rS = attp.tile([P, 1], F32, tag="rS")
                            nc.vector.reciprocal(rS[:], S_col[:])
                            rS_bf = attp.tile([P, 1], BF16, tag="rSb")
                            nc.vector.tensor_copy(rS_bf[:], rS[:])
                            ps_ctx = psc.tile([P, HD], F32)
                            nc.tensor.matmul(
                                ps_ctx[:], bdT[:],
                                v_sb[:, g * E + h * HD:g * E + (h + 1) * HD],
                                start=True, stop=True)
                            nc.scalar.activation(
                                ctx_sb[:, g * E + h * HD:g * E + (h + 1) * HD],
                                ps_ctx[:], AF.Copy, scale=rS[:, 0:1])
                            ps_u = psu.tile([P, 1], F32)
                            nc.tensor.matmul(ps_u[:], bd[:], rS_bf[:],
                                             start=True, stop=True)
                            nc.vector.tensor_tensor(
                                out=u_sb[:, g:g + 1], in0=u_sb[:, g:g + 1],
                                in1=ps_u[:], op=mybir.AluOpType.add)
                            if hp == NH // 2 - 1 and hh == 1:
                                # z = u @ ctx for group g, fused here so the
                                # LDW-bound z MMs overlap the attn pipeline.
                                # zT[e,s] = sum_p ctx[p,e]*(ind[p,s]*u[p]/256)
                                ind_u = attp.tile([P, GS], BF16, tag="iu")
                                nc.vector.tensor_scalar(
                                    out=ind_u[:], in0=ind_sb[:],
                                    scalar1=u_sb[:, g:g + 1],
                                    scalar2=1.0 / (L * NH),
                                    op0=mybir.AluOpType.mult,
                                    op1=mybir.AluOpType.mult)
                                for ec in range(KE):
                                    nc.tensor.matmul(
                                        ps_zT[:, ec * BC + g * GS:ec * BC + (g + 1) * GS],
                                        ctx_sb[:, g * E + ec * P:g * E + (ec + 1) * P],
                                        ind_u[:], start=True, stop=True)
                nc.vector.tensor_copy(zT_sb[:], ps_zT[:])

        # ---------------- MLP head (w_out folded into w1eff) ----------------
        with ExitStack() as s4:
            ps4 = s4.enter_context(tc.tile_pool(name="ps4", bufs=4, space="PSUM"))
            w1s3 = w1s[:].rearrange("p (ke n) -> p ke n", ke=KE)
            for nt in range(2):
                psum = ps4.tile([P, BC], F32, tag="mm")
                for ke in range(KE):
                    nc.tensor.matmul(psum[:], w1s3[:, ke, nt * P:(nt + 1) * P],
                                     zT_sb[:, ke * BC:(ke + 1) * BC],
                                     start=(ke == 0), stop=(ke == KE - 1))
                nc.scalar.activation(o1T_sb[:, nt * BC:(nt + 1) * BC], psum[:],
                                     AF.Relu, bias=b1T_sb[:, nt:nt + 1])
            psum = ps4.tile([P, BC], F32, tag="mm")
            w2s3 = w2_sb[:].rearrange("p (ke n) -> p ke n", ke=2)
            for ke in range(2):
                nc.tensor.matmul(psum[:], w2s3[:, ke, :],
                                 o1T_sb[:, ke * BC:(ke + 1) * BC],
                                 start=(ke == 0), stop=(ke == 1))
            nc.scalar.activation(o2T_sb[:], psum[:], AF.Relu,
                                 bias=b2T_sb[:, 0:1])
            psum3 = ps4.tile([P, BC], F32, tag="mm")
            nc.tensor.matmul(psum3[0:64, :], w3_sb[:], o2T_sb[:], start=True, stop=True)
            nc.scalar.activation(o3T_sb[:], psum3[0:64, :], AF.Relu, bias=b3T_sb[:, 0:1])
            psum4 = ps4.tile([P, BC], F32, tag="mm")
            nc.tensor.matmul(psum4[0:1, :], w4_sb[:], o3T_sb[:], start=True, stop=True)
            nc.scalar.activation(outT_sb[:], psum4[0:1, :], AF.Sigmoid,
                                 bias=b4_sb[:, 0:1])
            nc.sync.dma_start(out[:, 0:1], outT_sb[0:1, :])

    nc.compile()
    return nc


_BF = ml_dtypes.bfloat16
_F8 = ml_dtypes.float8_e4m3


def _q8(a, scale):
    return np.clip(np.asarray(a, np.float32) * scale, -240.0, 240.0).astype(_F8)


def _mbl():
    m = np.zeros((GS, P), np.float32)
    for j in range(GS):
        m[j, j * L:(j + 1) * L] = 1.0
    return m.astype(_BF)


def _mbr():
    m = np.full((GS, P), -1600.0, np.float32)
    for j in range(GS):
        m[j, j * L:(j + 1) * L] = 0.0
    return m.astype(_BF)


def _prep_shared(w_red, b_red, w_in, b_in, w_out, b_out, w1, b1, w2, b2, w3, b3,
                 w4, b4):
    f32 = np.float32
    w_red, w_in, w_out = (np.asarray(a, f32) for a in (w_red, w_in, w_out))
    w1 = np.asarray(w1, f32)
    b_in = np.asarray(b_in, f32)
    b_out_eff = np.asarray(b_out, f32) + w_out @ b_in[2 * E:3 * E]
    w1_eff = w1 @ w_out                     # (256, E)
    b1_eff = np.asarray(b1, f32) + w1 @ b_out_eff
    shared = {
        "wredT8": _q8(w_red.T, WRS),
        "winT8": _q8(w_in.T, WIS),
        "w1effT": np.ascontiguousarray(w1_eff.T).astype(_BF),
        "w2T": np.ascontiguousarray(np.asarray(w2, f32).T).astype(_BF),
        "w3T": np.ascontiguousarray(np.asarray(w3, f32).T).astype(_BF),
        "w4T": np.ascontiguousarray(np.asarray(w4, f32).T).astype(_BF),
        "bredT16": np.ascontiguousarray(
            (np.asarray(b_red, f32) * HS).reshape(KE, P).T),
        "binT": np.ascontiguousarray(b_in[:2 * E].reshape(2 * KE, P).T),
        "b1effT": np.ascontiguousarray(b1_eff.reshape(2, P).T),
        "b2T": np.ascontiguousarray(np.asarray(b2, f32).reshape(1, P).T),
        "b3T": np.ascontiguousarray(np.asarray(b3, f32).reshape(1, 64).T),
        "b4": np.asarray(b4, f32).reshape(1, 1),
        "mbl": _mbl(), "mbr": _mbr(),
    }
    return shared


def kernel(x, w_red, b_red, w_in, b_in, w_out, b_out, w1, b1, w2, b2, w3, b3,
           w4, b4):
    global LAST_EXEC_TIME_NS
    x = np.asarray(x, np.float32)
    shared = _prep_shared(w_red, b_red, w_in, b_in, w_out, b_out, w1, b1, w2,
                          b2, w3, b3, w4, b4)
    in_maps = []
    for c in range(NCORES):
        xc = x[c * BC:(c + 1) * BC].reshape(M, H)
        in_maps.append({"xT8": _q8(xc.T, XS), **shared})
    nc = _build_kernel()
    trace = os.environ.get("BASS_TRACE", "0") == "1"
    kw = {}
    if trace:
        _install_ntff_hook_shim()
        import concourse.bass_utils as _bu
        _bu.upload_artifacts = lambda d: str(d)  # no artifact bucket here
        tmpdir = os.environ.get("BASS_TRACE_DIR", "/tmp/bass_trace")
        os.makedirs(tmpdir, exist_ok=True)
        kw = {"trace": True, "tmpdir": tmpdir}
    res = run_bass_kernel_spmd(nc, in_maps, core_ids=list(range(NCORES)), **kw)
    LAST_EXEC_TIME_NS = res.exec_time_ns
    return np.concatenate([res.results[c]["out"] for c in range(NCORES)], axis=0)


if __name__ == "__main__":
    print("smoke test: building kernel only")
    _build_kernel()
    print("build OK")
